# revision 17
# baseline (speedup 1.0000x reference)
"""GAT (4-layer, softmax over dim=1) Trainium2 Bass kernel.

Sharding: data-parallel over batch B=8 -> one batch element per NeuronCore,
zero collectives. ~320 us HW exec, rel err ~2e-3 vs the jax reference.

T layout (j on partitions, i free): softmax over axis=1 is a free-axis
reduction; out[i,o] = sum_j att[j,i] Wh[j,o] takes attention strips as PE
operands. Per layer:

  Wh' = hT.T @ [W | W a1 | W a2]  (PE, fp16; f1/f2 ride along as 2 extra
        258-wide columns, eliminating 32 stationary-bound f12 matmuls/layer)
  f1b = broadcast f1 via wab matmul -> psum -> f16 SBUF (ACT copies)
  per j-strip [128 x 2048], software-pipelined (matmuls lag 2 strips):
    t = f1b + maskadd   (DVE tensor_tensor fp16, 2x mode; mask = {0,-500})
    prelu split by strip class to balance ACT vs DVE (both run ~50us/layer):
      class-A (strips 11-15): pm = Prelu(t + f2) on ACT (bias = f2 slice
        of Wh'; Prelu honors alpha)
      class-B (strips 0-10): v = 0.2t - 0.8 f2 (DVE tensor_scalar, 4x mode);
        w = max(t, v) (DVE, 2x); the +f2 rides the Exp bias
    expe = Exp(...) -> fp16, accum_out -> s  (ACT; shift-free softmax: e is
        bounded ~+/-5 and masked entries underflow to 0)
    whs = Wh[jt] * (1/s) -> fp16 (DVE); 8 PE matmuls accumulate
        outT[o,i] += whs.T @ expe into 8 PSUM banks
  tail: hT_next = Prelu(outT psum) on ACT; final layer transposes via PE
  identity matmuls and DMAs out fp32.

Measured no-gos for this problem: GpSimd tensor ops (only mult/add compile;
eff 0.42, and Q7 SBUF traffic halves co-running DVE throughput — slow DVE
ops correlate 87% with Pool activity); fp8e4 expe/whs with DoubleRow
matmuls (4x PE win, bit-exact matmul, but the HW fp8 casts truncate and
attention rows have small effective n, leaving a ~3% output bias — over the
2e-2 gate); tensor_scalar+accum_out (TensorScalarPtrReduce) runs at 1x, not
4x; scalar_tensor_tensor has no DVE perf modes.

Notes: walrus accepts at most one sync-wait per instruction
(split_multi_waits hoists extras); fp32 matmuls ~quarter fp16 rate; DVE
2x/4x perf modes need all-SBUF 2-byte packed operands; ACT rate is
dtype-independent (1 elem/partition/cycle at 1.2 GHz).
"""

import numpy as np
import ml_dtypes

import bass_rust
import concourse.bass as bass
import concourse.mybir as mybir
import concourse.tile as tile
from concourse.bass_utils import run_bass_kernel_spmd

f32 = mybir.dt.float32
f16 = mybir.dt.float16
fp8 = mybir.dt.float8e4
AFT = mybir.ActivationFunctionType
ALU = mybir.AluOpType
PM = mybir.MatmulPerfMode

B, N, F, L = 8, 2048, 256, 4
NT = N // 128  # 16 node tiles
FC = F // 128  # 2 feature chunks
IC = N // 512  # 4 i-chunks per strip
FW = F + 2     # 258: W columns + f1 + f2
ALPHA = 0.2
MASKADD = -500.0

ACT_PRELU = set(range(11, 16))  # class-A strips: prelu on ACT


def split_multi_waits(nc):
    """This container's walrus supports at most one sync-wait per instruction;
    Tile's exit drain (and occasionally the scheduler) attaches several. Hoist
    extras onto same-engine EventSemaphore instructions placed just before."""
    for fn in nc.m.functions:
        for blk in fn.blocks:
            new_list, changed = [], False
            for inst in blk.instructions:
                si = inst.sync_info
                if si is not None and len(si.on_wait) > 1:
                    waits = list(si.on_wait)
                    for k, w in enumerate(waits[:-1]):
                        es = mybir.InstEventSemaphore(name=f"{inst.name}_wsplit{k}")
                        es.engine = inst.engine
                        es.sync_info = bass_rust.SyncInfo(on_wait=[w], on_update=[])
                        new_list.append(es)
                    si.on_wait = [waits[-1]]
                    changed = True
                new_list.append(inst)
            if changed:
                blk.instructions = new_list


def build_nc(do_split=True):
    nc = bass.Bass()
    xT_d = nc.dram_tensor("xT", [F, N], f16, kind="ExternalInput")
    mask_d = nc.dram_tensor("maskT", [N, N], f16, kind="ExternalInput")
    W_d = nc.dram_tensor("W", [L, F, FW], f16, kind="ExternalInput")
    wab_d = nc.dram_tensor("wab", [L, F, 128], f16, kind="ExternalInput")
    sinv_d = nc.dram_tensor("sinv", [128, L], f32, kind="ExternalInput")
    ident_d = nc.dram_tensor("ident", [128, 128], f32, kind="ExternalInput")
    out_d = nc.dram_tensor("out", [N, F], f32, kind="ExternalOutput")

    # host-chosen 1/S_l compile-time constants are not possible (data dep),
    # so S_l is fixed analytically on the host and 1/S_l passed via sinv.
    with tile.TileContext(nc) as tc:
        with (
            tc.tile_pool(name="const", bufs=1) as constp,
            tc.tile_pool(name="hT", bufs=2) as hTp,
            tc.tile_pool(name="wl", bufs=2) as wlp,
            tc.tile_pool(name="wh", bufs=2) as whp,
            tc.tile_pool(name="eb", bufs=2) as ebp,
            tc.tile_pool(name="strip", bufs=5) as stripp,
            tc.tile_pool(name="mp", bufs=5) as mpp,
            tc.tile_pool(name="sr", bufs=10) as srp,
            tc.tile_pool(name="whs", bufs=5) as whsp,
            tc.tile_pool(name="outsb", bufs=3) as outp,
            tc.tile_pool(name="bank", bufs=8, space="PSUM") as psp,
        ):
            ident_sb = constp.tile([128, 128], f32)
            nc.sync.dma_start(ident_sb[:], ident_d[:])
            sinv_sb = constp.tile([128, L], f32)
            nc.sync.dma_start(sinv_sb[:], sinv_d[:])
            hT_cur = hTp.tile([128, FC * N], f16, tag="hT")
            for fc in range(FC):
                nc.sync.dma_start(
                    hT_cur[:, fc * N : (fc + 1) * N],
                    xT_d[fc * 128 : (fc + 1) * 128, :],
                )

            def load_layer_weights(l):
                W_sb = wlp.tile([128, FC * FW], f16, tag="W", name=f"W_{l}")
                wab_sb = wlp.tile([128, FC * 128], f16, tag="wab", name=f"wab_{l}")
                for fc in range(FC):
                    nc.sync.dma_start(
                        W_sb[:, fc * FW : (fc + 1) * FW],
                        W_d[l, fc * 128 : (fc + 1) * 128, :],
                    )
                    nc.sync.dma_start(
                        wab_sb[:, fc * 128 : (fc + 1) * 128],
                        wab_d[l, fc * 128 : (fc + 1) * 128, :],
                    )
                return W_sb, wab_sb

            weights0 = load_layer_weights(0)
            mask_sb = constp.tile([128, NT * N], f16)
            for jt in range(NT):
                nc.sync.dma_start(
                    mask_sb[:, jt * N : (jt + 1) * N],
                    mask_d[jt * 128 : (jt + 1) * 128, :],
                )

            for l in range(L):
                if l == 0:
                    W_sb, wab_sb = weights0
                else:
                    W_sb, wab_sb = load_layer_weights(l)

                # ---- Wh' (f1/f2 side columns ride along) ----
                Wh_sb = whp.tile([128, NT * FW], f16, tag="Wh", name=f"Wh_{l}")
                for nt in range(NT):
                    ps = psp.tile([128, 512], f32, tag="bank")
                    for fc in range(FC):
                        nc.tensor.matmul(
                            ps[:, 0:FW],
                            hT_cur[:, fc * N + nt * 128 : fc * N + (nt + 1) * 128],
                            W_sb[:, fc * FW : (fc + 1) * FW],
                            start=(fc == 0),
                            stop=(fc == FC - 1),
                        )
                    nc.scalar.copy(Wh_sb[:, nt * FW : (nt + 1) * FW], ps[:, 0:FW])

                # negf2 = -0.8*f2 for the class-B tensor_scalar
                negf2 = srp.tile([128, NT], f32, tag="negf2", name=f"nf2_{l}")
                nc.vector.tensor_scalar_mul(
                    negf2[:, :], Wh_sb[:, F + 1 :: FW], -0.8
                )

                # ---- f1 broadcast -> f16 SBUF ----
                f1b_sb = ebp.tile([128, N], f16, tag="f1b", name=f"f1b_{l}")
                for ic in range(IC):
                    ps = psp.tile([128, 512], f32, tag="bank")
                    for fc in range(FC):
                        nc.tensor.matmul(
                            ps[:, :],
                            wab_sb[:, fc * 128 : (fc + 1) * 128],
                            hT_cur[:, fc * N + ic * 512 : fc * N + (ic + 1) * 512],
                            start=(fc == 0),
                            stop=(fc == FC - 1),
                        )
                    nc.scalar.copy(f1b_sb[:, ic * 512 : (ic + 1) * 512], ps[:, :])

                # ---- strip loop in pairs ----
                psum_out = [
                    psp.tile([128, 512], f32, tag="bank", name=f"po_{l}_{k}")
                    for k in range(8)
                ]
                m_t = [None] * NT
                whs_t = [None] * NT

                def emit_front(jt):
                    m_t[jt] = mpp.tile([128, N], f16, tag="mp", name=f"mp_{l}_{jt}")
                    whs_t[jt] = whsp.tile([128, F], f16, tag="wp", name=f"wp_{l}_{jt}")
                    t = stripp.tile([128, N], f16, tag="t", name=f"t_{l}_{jt}")
                    nc.vector.tensor_tensor(
                        t[:, :],
                        f1b_sb[:, :],
                        mask_sb[:, jt * N : (jt + 1) * N],
                        ALU.add,
                    )
                    f2_ap = Wh_sb[:, jt * FW + F + 1 : jt * FW + F + 2]
                    expe = m_t[jt][:, :]
                    s = srp.tile([128, 1], f32, tag="s", name=f"s_{l}_{jt}")
                    if jt in ACT_PRELU:
                        nc.scalar.activation(
                            t[:, :], t[:, :], AFT.Prelu,
                            bias=f2_ap, scale=1.0, alpha=ALPHA,
                        )
                        nc.scalar.activation(
                            expe, t[:, :], AFT.Exp, accum_out=s[:, :]
                        )
                    else:
                        v = stripp.tile([128, N], f16, tag="v", name=f"v_{l}_{jt}")
                        nc.vector.tensor_scalar(
                            v[:, :], t[:, :], 0.2, negf2[:, jt : jt + 1],
                            ALU.mult, ALU.add,
                        )
                        nc.vector.tensor_tensor(t[:, :], t[:, :], v[:, :], ALU.max)
                        nc.scalar.activation(
                            expe, t[:, :], AFT.Exp,
                            bias=f2_ap, accum_out=s[:, :],
                        )
                    r_t = srp.tile([128, 1], f32, tag="r", name=f"r_{l}_{jt}")
                    nc.vector.reciprocal(r_t[:, :], s[:, :])
                    nc.vector.tensor_scalar_mul(
                        whs_t[jt][:, :],
                        Wh_sb[:, jt * FW : jt * FW + F],
                        r_t[:, :],
                    )

                def emit_back(jt):
                    for oc in range(2):
                        for ic in range(IC):
                            nc.tensor.matmul(
                                psum_out[oc * IC + ic][:, :],
                                whs_t[jt][:, oc * 128 : (oc + 1) * 128],
                                m_t[jt][:, ic * 512 : (ic + 1) * 512],
                                start=(jt == 0),
                                stop=(jt == NT - 1),
                            )

                for jt in range(NT + 2):
                    if jt >= 2:
                        emit_back(jt - 2)
                    if jt < NT:
                        emit_front(jt)

                # ---- tail (undoes the host S_l scale via 1/S_l) ----
                if l < L - 1:
                    hT_next = hTp.tile([128, FC * N], f16, tag="hT")
                else:
                    hT_next = hTp.tile([128, FC * N], f32, tag="hTf32", bufs=1)
                for ic in range(IC):
                    for oc in range(2):
                        dst = hT_next[:, oc * N + ic * 512 : oc * N + (ic + 1) * 512]
                        ps = psum_out[oc * IC + ic]
                        nc.scalar.activation(
                            dst, ps[:, :], AFT.Prelu,
                            scale=sinv_sb[:, l : l + 1],
                            alpha=ALPHA,
                        )
                if l < L - 1:
                    hT_cur = hT_next
                else:
                    # transpose houtT [o, i] -> out [i, o] via PE identity matmuls
                    for nt in range(NT):
                        ob = outp.tile([128, F], f32, tag="ob")
                        for oc in range(FC):
                            pst = psp.tile([128, 512], f32, tag="bank", name=f"tr_{nt}_{oc}")
                            nc.tensor.matmul(
                                pst[:, 0:128],
                                hT_next[:, oc * N + nt * 128 : oc * N + (nt + 1) * 128],
                                ident_sb[:, :],
                                start=True,
                                stop=True,
                            )
                            if oc % 2 == 0:
                                nc.scalar.copy(
                                    ob[:, oc * 128 : (oc + 1) * 128], pst[:, 0:128]
                                )
                            else:
                                nc.vector.tensor_copy(
                                    ob[:, oc * 128 : (oc + 1) * 128], pst[:, 0:128]
                                )
                        nc.sync.dma_start(out_d[nt * 128 : (nt + 1) * 128, :], ob[:, :])

    if do_split:
        split_multi_waits(nc)
    return nc


_NC = None


def _get_nc():
    global _NC
    if _NC is None:
        _NC = build_nc()
    return _NC


def _layer_scales(W_all, A):
    """Analytic per-layer scale S_l so whs = S_l*Wh/s sits near 1.0 in fp8
    (fp8e4 tolerates ~100x misestimate either way). Attention averaging only
    shrinks activations at layer 0 -- afterwards node features are strongly
    correlated (every h_j is a near-identical neighborhood mean), so later
    layers keep roughly constant scale. S is clamped so S*W fits fp16."""
    S = []
    hstd = 1.0
    for l in range(L):
        whstd = float(np.std(W_all[l])) * np.sqrt(F) * hstd
        a_norm = float(np.linalg.norm(A[l, :F]))
        fstd = a_norm * whstd
        zvar = 2.0 * fstd * fstd
        s_est = (N / 2) * np.exp(min(zvar, 8.0) / 2.0)
        # fp16 whs/expe (as in the proven baseline) need no rescaling
        S.append(1.0)
        s_est = s_est
        if l == 0:
            sum_att2 = np.exp(min(zvar, 8.0)) / (N / 2)
            hstd = whstd * np.sqrt(sum_att2) * 0.72
        else:
            hstd *= 0.75
    return S


def _host_prep(x, adj, W0, Wrest, A):
    x = np.asarray(x, dtype=np.float32)
    adj = np.asarray(adj)
    W_all = np.stack(
        [np.asarray(W0, dtype=np.float32)]
        + [np.asarray(Wrest[i], dtype=np.float32) for i in range(L - 1)]
    )  # [4, F, F]
    A = np.asarray(A, dtype=np.float32)
    S = _layer_scales(W_all, A)
    W258 = np.empty((L, F, FW), dtype=np.float32)
    wab = np.empty((L, F, 128), dtype=np.float32)
    for l in range(L):
        W258[l, :, F] = W_all[l] @ A[l, :F]       # f1 weights (unscaled)
        W258[l, :, F + 1] = W_all[l] @ A[l, F:]   # f2 weights (unscaled)
        W258[l, :, :F] = W_all[l] * S[l]
        wab[l] = np.repeat(W258[l, :, F : F + 1], 128, axis=1)
    ident = np.eye(128, dtype=np.float32)
    sinv = np.repeat(np.array([[1.0 / s for s in S]], dtype=np.float32), 128, axis=0)
    W_16 = W258.astype(np.float16)
    wab_16 = wab.astype(np.float16)

    in_maps = []
    for b in range(B):
        xT = np.ascontiguousarray(x[b].T).astype(np.float16)
        adjT = adj[b].T.astype(np.float32)
        maskT = ((adjT - 1.0) * (-MASKADD)).astype(np.float16)
        in_maps.append(
            {
                "xT": xT,
                "maskT": maskT,
                "W": W_16,
                "wab": wab_16,
                "sinv": sinv,
                "ident": ident,
            }
        )
    return in_maps


def kernel(x, adj, W0, Wrest, A, _trace=False, _trace_kwargs=None):
    nc = _get_nc()
    in_maps = _host_prep(x, adj, W0, Wrest, A)
    res = run_bass_kernel_spmd(
        nc,
        in_maps,
        core_ids=list(range(B)),
        trace=_trace,
        **(_trace_kwargs or {}),
    )
    out = np.stack([res.results[b]["out"] for b in range(B)])
    if _trace:
        kernel.last_exec_time_ns = res.exec_time_ns
        kernel.last_results = res
    return out


# revision 19
# speedup vs baseline: 1.0108x; 1.0108x over previous
"""GAT (4-layer, softmax over dim=1) Trainium2 Bass kernel.

Sharding: data-parallel over batch B=8 -> one batch element per NeuronCore,
zero collectives. ~320 us HW exec, rel err ~2e-3 vs the jax reference.

T layout (j on partitions, i free): softmax over axis=1 is a free-axis
reduction; out[i,o] = sum_j att[j,i] Wh[j,o] takes attention strips as PE
operands. Per layer:

  Wh' = hT.T @ [W | W a1 | W a2]  (PE, fp16; f1/f2 ride along as 2 extra
        258-wide columns, eliminating 32 stationary-bound f12 matmuls/layer)
  f1b = broadcast f1 via wab matmul -> psum -> f16 SBUF (ACT copies)
  per j-strip [128 x 2048], software-pipelined (matmuls lag 2 strips):
    t = f1b + maskadd   (DVE tensor_tensor fp16, 2x mode; mask = {0,-500})
    prelu split by strip class to balance ACT vs DVE (both run ~50us/layer):
      class-A (strips 11-15): pm = Prelu(t + f2) on ACT (bias = f2 slice
        of Wh'; Prelu honors alpha)
      class-B (strips 0-10): v = 0.2t - 0.8 f2 (DVE tensor_scalar, 4x mode);
        w = max(t, v) (DVE, 2x); the +f2 rides the Exp bias
    expe = Exp(...) -> fp16, accum_out -> s  (ACT; shift-free softmax: e is
        bounded ~+/-5 and masked entries underflow to 0)
    whs = Wh[jt] * (1/s) -> fp16 (DVE); 8 PE matmuls accumulate
        outT[o,i] += whs.T @ expe into 8 PSUM banks
  tail: hT_next = Prelu(outT psum) on ACT; final layer transposes via PE
  identity matmuls and DMAs out fp32.

Measured no-gos for this problem: GpSimd tensor ops (only mult/add compile;
eff 0.42, and Q7 SBUF traffic halves co-running DVE throughput — slow DVE
ops correlate 87% with Pool activity); fp8e4 expe/whs with DoubleRow
matmuls (4x PE win, bit-exact matmul, but the HW fp8 casts truncate and
attention rows have small effective n, leaving a ~3% output bias — over the
2e-2 gate); tensor_scalar+accum_out (TensorScalarPtrReduce) runs at 1x, not
4x; scalar_tensor_tensor has no DVE perf modes.

Notes: walrus accepts at most one sync-wait per instruction
(split_multi_waits hoists extras); fp32 matmuls ~quarter fp16 rate; DVE
2x/4x perf modes need all-SBUF 2-byte packed operands; ACT rate is
dtype-independent (1 elem/partition/cycle at 1.2 GHz).
"""

import numpy as np
import ml_dtypes

import bass_rust
import concourse.bass as bass
import concourse.mybir as mybir
import concourse.tile as tile
from concourse.bass_utils import run_bass_kernel_spmd

f32 = mybir.dt.float32
f16 = mybir.dt.float16
fp8 = mybir.dt.float8e4
AFT = mybir.ActivationFunctionType
ALU = mybir.AluOpType
PM = mybir.MatmulPerfMode

B, N, F, L = 8, 2048, 256, 4
NT = N // 128  # 16 node tiles
FC = F // 128  # 2 feature chunks
IC = N // 512  # 4 i-chunks per strip
FW = F + 2     # 258: W columns + f1 + f2
ALPHA = 0.2
MASKADD = -500.0

ACT_PRELU = set(range(12, 16))  # class-A strips: prelu on ACT


def split_multi_waits(nc):
    """This container's walrus supports at most one sync-wait per instruction;
    Tile's exit drain (and occasionally the scheduler) attaches several. Hoist
    extras onto same-engine EventSemaphore instructions placed just before."""
    for fn in nc.m.functions:
        for blk in fn.blocks:
            new_list, changed = [], False
            for inst in blk.instructions:
                si = inst.sync_info
                if si is not None and len(si.on_wait) > 1:
                    waits = list(si.on_wait)
                    for k, w in enumerate(waits[:-1]):
                        es = mybir.InstEventSemaphore(name=f"{inst.name}_wsplit{k}")
                        es.engine = inst.engine
                        es.sync_info = bass_rust.SyncInfo(on_wait=[w], on_update=[])
                        new_list.append(es)
                    si.on_wait = [waits[-1]]
                    changed = True
                new_list.append(inst)
            if changed:
                blk.instructions = new_list


def build_nc(do_split=True):
    nc = bass.Bass()
    xT_d = nc.dram_tensor("xT", [F, N], f16, kind="ExternalInput")
    mask_d = nc.dram_tensor("maskT", [N, N], f16, kind="ExternalInput")
    W_d = nc.dram_tensor("W", [L, F, FW], f16, kind="ExternalInput")
    wab_d = nc.dram_tensor("wab", [L, F, 128], f16, kind="ExternalInput")
    sinv_d = nc.dram_tensor("sinv", [128, L], f32, kind="ExternalInput")
    ident_d = nc.dram_tensor("ident", [128, 128], f32, kind="ExternalInput")
    out_d = nc.dram_tensor("out", [N, F], f32, kind="ExternalOutput")

    # host-chosen 1/S_l compile-time constants are not possible (data dep),
    # so S_l is fixed analytically on the host and 1/S_l passed via sinv.
    with tile.TileContext(nc) as tc:
        with (
            tc.tile_pool(name="const", bufs=1) as constp,
            tc.tile_pool(name="hT", bufs=2) as hTp,
            tc.tile_pool(name="wl", bufs=2) as wlp,
            tc.tile_pool(name="wh", bufs=2) as whp,
            tc.tile_pool(name="eb", bufs=2) as ebp,
            tc.tile_pool(name="strip", bufs=3) as stripp,
            tc.tile_pool(name="mp", bufs=3) as mpp,
            tc.tile_pool(name="sr", bufs=6) as srp,
            tc.tile_pool(name="whs", bufs=3) as whsp,
            tc.tile_pool(name="outsb", bufs=3) as outp,
            tc.tile_pool(name="bank", bufs=8, space="PSUM") as psp,
        ):
            ident_sb = constp.tile([128, 128], f32)
            nc.sync.dma_start(ident_sb[:], ident_d[:])
            sinv_sb = constp.tile([128, L], f32)
            nc.sync.dma_start(sinv_sb[:], sinv_d[:])
            hT_cur = hTp.tile([128, FC * N], f16, tag="hT")
            for fc in range(FC):
                nc.sync.dma_start(
                    hT_cur[:, fc * N : (fc + 1) * N],
                    xT_d[fc * 128 : (fc + 1) * 128, :],
                )

            def load_layer_weights(l):
                W_sb = wlp.tile([128, FC * FW], f16, tag="W", name=f"W_{l}")
                wab_sb = wlp.tile([128, FC * 128], f16, tag="wab", name=f"wab_{l}")
                for fc in range(FC):
                    nc.sync.dma_start(
                        W_sb[:, fc * FW : (fc + 1) * FW],
                        W_d[l, fc * 128 : (fc + 1) * 128, :],
                    )
                    nc.sync.dma_start(
                        wab_sb[:, fc * 128 : (fc + 1) * 128],
                        wab_d[l, fc * 128 : (fc + 1) * 128, :],
                    )
                return W_sb, wab_sb

            weights0 = load_layer_weights(0)
            mask_sb = constp.tile([128, NT * N], f16)
            for jt in range(NT):
                nc.sync.dma_start(
                    mask_sb[:, jt * N : (jt + 1) * N],
                    mask_d[jt * 128 : (jt + 1) * 128, :],
                )

            for l in range(L):
                if l == 0:
                    W_sb, wab_sb = weights0
                else:
                    W_sb, wab_sb = load_layer_weights(l)

                # ---- Wh' (f1/f2 side columns ride along) ----
                Wh_sb = whp.tile([128, NT * FW], f16, tag="Wh", name=f"Wh_{l}")
                for nt in range(NT):
                    ps = psp.tile([128, 512], f32, tag="bank")
                    for fc in range(FC):
                        nc.tensor.matmul(
                            ps[:, 0:FW],
                            hT_cur[:, fc * N + nt * 128 : fc * N + (nt + 1) * 128],
                            W_sb[:, fc * FW : (fc + 1) * FW],
                            start=(fc == 0),
                            stop=(fc == FC - 1),
                        )
                    nc.scalar.copy(Wh_sb[:, nt * FW : (nt + 1) * FW], ps[:, 0:FW])

                # negf2 = -0.8*f2 for the class-B tensor_scalar
                negf2 = srp.tile([128, NT], f32, tag="negf2", name=f"nf2_{l}")
                nc.vector.tensor_scalar_mul(
                    negf2[:, :], Wh_sb[:, F + 1 :: FW], -0.8
                )

                # ---- f1 broadcast -> f16 SBUF ----
                f1b_sb = ebp.tile([128, N], f16, tag="f1b", name=f"f1b_{l}")
                for ic in range(IC):
                    ps = psp.tile([128, 512], f32, tag="bank")
                    for fc in range(FC):
                        nc.tensor.matmul(
                            ps[:, :],
                            wab_sb[:, fc * 128 : (fc + 1) * 128],
                            hT_cur[:, fc * N + ic * 512 : fc * N + (ic + 1) * 512],
                            start=(fc == 0),
                            stop=(fc == FC - 1),
                        )
                    nc.scalar.copy(f1b_sb[:, ic * 512 : (ic + 1) * 512], ps[:, :])

                # ---- strip loop in pairs ----
                psum_out = [
                    psp.tile([128, 512], f32, tag="bank", name=f"po_{l}_{k}")
                    for k in range(8)
                ]
                m_t = [None] * NT
                whs_t = [None] * NT

                def emit_front(jt):
                    m_t[jt] = mpp.tile([128, N], f16, tag="mp", name=f"mp_{l}_{jt}")
                    whs_t[jt] = whsp.tile([128, F], f16, tag="wp", name=f"wp_{l}_{jt}")
                    t = stripp.tile([128, N], f16, tag="t", name=f"t_{l}_{jt}")
                    nc.vector.tensor_tensor(
                        t[:, :],
                        f1b_sb[:, :],
                        mask_sb[:, jt * N : (jt + 1) * N],
                        ALU.add,
                    )
                    f2_ap = Wh_sb[:, jt * FW + F + 1 : jt * FW + F + 2]
                    expe = m_t[jt][:, :]
                    s = srp.tile([128, 1], f32, tag="s", name=f"s_{l}_{jt}")
                    if jt in ACT_PRELU:
                        nc.scalar.activation(
                            t[:, :], t[:, :], AFT.Prelu,
                            bias=f2_ap, scale=1.0, alpha=ALPHA,
                        )
                        nc.scalar.activation(
                            expe, t[:, :], AFT.Exp, accum_out=s[:, :]
                        )
                    else:
                        v = stripp.tile([128, N], f16, tag="v", name=f"v_{l}_{jt}")
                        nc.vector.tensor_scalar(
                            v[:, :], t[:, :], 0.2, negf2[:, jt : jt + 1],
                            ALU.mult, ALU.add,
                        )
                        nc.vector.tensor_tensor(t[:, :], t[:, :], v[:, :], ALU.max)
                        nc.scalar.activation(
                            expe, t[:, :], AFT.Exp,
                            bias=f2_ap, accum_out=s[:, :],
                        )
                    r_t = srp.tile([128, 1], f32, tag="r", name=f"r_{l}_{jt}")
                    nc.vector.reciprocal(r_t[:, :], s[:, :])
                    nc.vector.tensor_scalar_mul(
                        whs_t[jt][:, :],
                        Wh_sb[:, jt * FW : jt * FW + F],
                        r_t[:, :],
                    )

                def emit_back(jt):
                    for oc in range(2):
                        for ic in range(IC):
                            nc.tensor.matmul(
                                psum_out[oc * IC + ic][:, :],
                                whs_t[jt][:, oc * 128 : (oc + 1) * 128],
                                m_t[jt][:, ic * 512 : (ic + 1) * 512],
                                start=(jt == 0),
                                stop=(jt == NT - 1),
                            )

                for jt in range(NT + 2):
                    if jt >= 2:
                        emit_back(jt - 2)
                    if jt < NT:
                        emit_front(jt)

                # ---- tail (undoes the host S_l scale via 1/S_l) ----
                if l < L - 1:
                    hT_next = hTp.tile([128, FC * N], f16, tag="hT")
                else:
                    hT_next = hTp.tile([128, FC * N], f32, tag="hTf32", bufs=1)
                for ic in range(IC):
                    for oc in range(2):
                        dst = hT_next[:, oc * N + ic * 512 : oc * N + (ic + 1) * 512]
                        ps = psum_out[oc * IC + ic]
                        nc.scalar.activation(
                            dst, ps[:, :], AFT.Prelu,
                            scale=sinv_sb[:, l : l + 1],
                            alpha=ALPHA,
                        )
                if l < L - 1:
                    hT_cur = hT_next
                else:
                    # transpose houtT [o, i] -> out [i, o] via PE identity matmuls
                    for nt in range(NT):
                        ob = outp.tile([128, F], f32, tag="ob")
                        for oc in range(FC):
                            pst = psp.tile([128, 512], f32, tag="bank", name=f"tr_{nt}_{oc}")
                            nc.tensor.matmul(
                                pst[:, 0:128],
                                hT_next[:, oc * N + nt * 128 : oc * N + (nt + 1) * 128],
                                ident_sb[:, :],
                                start=True,
                                stop=True,
                            )
                            if oc % 2 == 0:
                                nc.scalar.copy(
                                    ob[:, oc * 128 : (oc + 1) * 128], pst[:, 0:128]
                                )
                            else:
                                nc.vector.tensor_copy(
                                    ob[:, oc * 128 : (oc + 1) * 128], pst[:, 0:128]
                                )
                        nc.sync.dma_start(out_d[nt * 128 : (nt + 1) * 128, :], ob[:, :])

    if do_split:
        split_multi_waits(nc)
    return nc


_NC = None


def _get_nc():
    global _NC
    if _NC is None:
        _NC = build_nc()
    return _NC


def _layer_scales(W_all, A):
    """Analytic per-layer scale S_l so whs = S_l*Wh/s sits near 1.0 in fp8
    (fp8e4 tolerates ~100x misestimate either way). Attention averaging only
    shrinks activations at layer 0 -- afterwards node features are strongly
    correlated (every h_j is a near-identical neighborhood mean), so later
    layers keep roughly constant scale. S is clamped so S*W fits fp16."""
    S = []
    hstd = 1.0
    for l in range(L):
        whstd = float(np.std(W_all[l])) * np.sqrt(F) * hstd
        a_norm = float(np.linalg.norm(A[l, :F]))
        fstd = a_norm * whstd
        zvar = 2.0 * fstd * fstd
        s_est = (N / 2) * np.exp(min(zvar, 8.0) / 2.0)
        # fp16 whs/expe (as in the proven baseline) need no rescaling
        S.append(1.0)
        s_est = s_est
        if l == 0:
            sum_att2 = np.exp(min(zvar, 8.0)) / (N / 2)
            hstd = whstd * np.sqrt(sum_att2) * 0.72
        else:
            hstd *= 0.75
    return S


def _host_prep(x, adj, W0, Wrest, A):
    x = np.asarray(x, dtype=np.float32)
    adj = np.asarray(adj)
    W_all = np.stack(
        [np.asarray(W0, dtype=np.float32)]
        + [np.asarray(Wrest[i], dtype=np.float32) for i in range(L - 1)]
    )  # [4, F, F]
    A = np.asarray(A, dtype=np.float32)
    S = _layer_scales(W_all, A)
    W258 = np.empty((L, F, FW), dtype=np.float32)
    wab = np.empty((L, F, 128), dtype=np.float32)
    for l in range(L):
        W258[l, :, F] = W_all[l] @ A[l, :F]       # f1 weights (unscaled)
        W258[l, :, F + 1] = W_all[l] @ A[l, F:]   # f2 weights (unscaled)
        W258[l, :, :F] = W_all[l] * S[l]
        wab[l] = np.repeat(W258[l, :, F : F + 1], 128, axis=1)
    ident = np.eye(128, dtype=np.float32)
    sinv = np.repeat(np.array([[1.0 / s for s in S]], dtype=np.float32), 128, axis=0)
    W_16 = W258.astype(np.float16)
    wab_16 = wab.astype(np.float16)

    in_maps = []
    for b in range(B):
        xT = np.ascontiguousarray(x[b].T).astype(np.float16)
        adjT = adj[b].T.astype(np.float32)
        maskT = ((adjT - 1.0) * (-MASKADD)).astype(np.float16)
        in_maps.append(
            {
                "xT": xT,
                "maskT": maskT,
                "W": W_16,
                "wab": wab_16,
                "sinv": sinv,
                "ident": ident,
            }
        )
    return in_maps


def kernel(x, adj, W0, Wrest, A, _trace=False, _trace_kwargs=None):
    nc = _get_nc()
    in_maps = _host_prep(x, adj, W0, Wrest, A)
    res = run_bass_kernel_spmd(
        nc,
        in_maps,
        core_ids=list(range(B)),
        trace=_trace,
        **(_trace_kwargs or {}),
    )
    out = np.stack([res.results[b]["out"] for b in range(B)])
    if _trace:
        kernel.last_exec_time_ns = res.exec_time_ns
        kernel.last_results = res
    return out
